# revision 25
# baseline (speedup 1.0000x reference)
"""GCNConv (rank-1 normalized aggregation) Trainium2 kernel, SPMD over 8 cores.

Math (faithful to the torch/jax reference):
    h    = x @ W
    adj  = symmetric 0/1 adjacency from edge_index (duplicates collapse: SET, not add)
    deg  = adj.sum(1);  dinv = 1/sqrt(deg)
    agg  = dinv @ h                      # rank-1 identity, [F_OUT]
    out  = dinv[:, None] * agg[None, :] + bias

Since agg = (dinv @ x) @ W, h is never materialized:
    v    = dinv @ x            ([F_IN] weighted row-sum)
    agg  = v @ W               (TensorE)
    out_c = dinv_c (x) agg + bias     (rows sharded across cores)

Collectives here have a ~55us fixed latency, far above the 8-core floor, so
every core reads the full x (3.07MB bf16, ~9us at HBM BW) and computes v
locally; only the O(N*F_OUT) output is sharded.

v runs entirely on TensorE: per 128-node slice r, the dinv column [128,1] is
the stationary operand (LDWEIGHTS cost scales with stationary *columns*, so a
1-column load is ~1 cycle) and the raw bf16 x slice [128,128] is the moving
operand; all 96 matmuls accumulate into one [1,128] PSUM tile. No DVE
pre-multiply, no fat ones-matmul.

DMA shape discipline (measured): each 128-descriptor dma_start costs ~680ns
of HWDGE sequencer issue time and each transfer's completion semaphore fires
~1.5-2.5us after its last byte (receipt latency), so x travels in 6 big
r-grouped transfers (24KB contiguous per partition total), ALL on the sync
ring in consumption order (so sems fire in the order TensorE consumes),
with small first group (early TensorE start) and small last group (short
post-receipt matmul backlog). Consts ride the scalar ring. ~3us of dummy
matmuls pre-warm the PE HAM clock gate to 8/8 during the DMA lead-in.
Measured fixed costs: ~7us engine preamble before the first DMA issue and
~3.5us semaphore teardown after the last write - a trivial kernel measures
13.4us on this stack, so ~29us total = floor + 9us x stream + tails.

The exact deduplicated degree (an integer/sorting problem, not a flops
problem) is computed on host with np.unique; all O(N*F) floating-point work
runs on the NeuronCores. Output travels bf16 and is upcast on host.
"""

import numpy as np

N, F_IN, F_OUT = 12000, 128, 256
N_CORES = 8
ROWS = N // N_CORES            # 1500 output rows per core
NT_OUT = 12                    # rows per partition in the output shard
ROWS_PAD = NT_OUT * 128        # 1536
R_TOT = 96                     # x rows per partition (node = p*96 + r)
N_PAD = 128 * R_TOT            # 12288
X_GROUPS = [4, 24, 28, 24, 8, 4, 4]
DVE_ROWS = 9                  # expansion rows on DVE (tensor_scalar_mul)
ACT_ROWS = NT_OUT - DVE_ROWS   # expansion rows on ScalarE (activation scale)
C_DT = R_TOT + NT_OUT          # packed bf16 consts: dinvT | dinvS

_cache = {}


def _build_nc(zero_bias):
    import concourse.bacc as bacc
    import concourse.mybir as mybir
    import concourse.tile as tile

    f32 = mybir.dt.float32
    bf16 = mybir.dt.bfloat16

    nc = bacc.Bacc(
        "TRN2",
        target_bir_lowering=False,
        debug=False,
        num_devices=N_CORES,
    )

    # x padded to [12288, 128] bf16; partition p holds rows p*96 .. p*96+95
    x_d = nc.dram_tensor("x", [N_PAD, F_IN], bf16, kind="ExternalInput")
    # cb16[:, 0:96] = dinvT (dinvT[p, r] = dinv[p*96+r]);
    # cb16[:, 96:108] = dinvS (dinvS[p, i] = dinv[core*1500 + p*12 + i])
    cb16_d = nc.dram_tensor("cb16", [128, C_DT], bf16, kind="ExternalInput")
    # f32 copy of dinvS for the ScalarE activation scale operand
    dinvSf_d = nc.dram_tensor("dinvSf", [128, NT_OUT], f32, kind="ExternalInput")
    w_d = nc.dram_tensor("weight", [F_IN, F_OUT], bf16, kind="ExternalInput")
    if not zero_bias:
        b_d = nc.dram_tensor("bias", [F_OUT], f32, kind="ExternalInput")
    out_d = nc.dram_tensor("out", [ROWS_PAD, F_OUT], bf16, kind="ExternalOutput")

    x_prm = x_d.ap().rearrange("(p r) m -> p r m", p=128)       # [128,96,128]
    # out row p*12 + n  ->  partition p, free n  (natural row-major)
    out_view = out_d.ap().rearrange("(p n) m -> p n m", p=128)  # [128,12,256]

    with tile.TileContext(nc) as tc:
        with (
            tc.tile_pool(name="const", bufs=1) as cpool,
            tc.tile_pool(name="xbuf", bufs=1) as xpool,
            tc.tile_pool(name="obuf", bufs=1) as opool,
            tc.tile_pool(name="ps", bufs=1, space="PSUM") as psum,
        ):
            # ---- DMAs: x groups alternate the two HWDGE queues; the packed
            # consts lead the scalar queue so TensorE can start on group 0 ----
            cb16 = cpool.tile([128, C_DT], bf16)
            nc.scalar.dma_start(cb16[:], cb16_d.ap())
            dinvT = cb16[:, 0:R_TOT]
            dinvS = cb16[:, R_TOT : R_TOT + NT_OUT]

            xg = []
            r0 = 0
            x_offs = []
            for g, rsz in enumerate(X_GROUPS):
                t = xpool.tile([128, rsz, F_IN], bf16, tag=f"xg{g}",
                               name=f"xg{g}")
                # all x on the sync ring, in consumption order: bytes (and so
                # completion sems) arrive exactly in the order TensorE needs
                nc.sync.dma_start(t[:], x_prm[:, r0 : r0 + rsz, :])
                xg.append(t)
                x_offs.append(r0)
                r0 += rsz

            dinvSf = cpool.tile([128, NT_OUT], f32)
            nc.scalar.dma_start(dinvSf[:], dinvSf_d.ap())
            w_s = cpool.tile([F_IN, F_OUT], bf16)
            nc.scalar.dma_start(w_s[:], w_d.ap())
            if not zero_bias:
                bias_s = cpool.tile([1, F_OUT], f32)
                nc.scalar.dma_start(
                    bias_s[:], b_d.ap().rearrange("(a n) -> a n", a=1)
                )
                ones_row = cpool.tile([1, 128], f32)
                nc.vector.memset(ones_row[:], 1.0)
            one_s = cpool.tile([1, 1], f32)
            nc.vector.memset(one_s[:], 1.0)

            # ---- PE pre-warm: ~3us of dummy matmuls while the x DMA is in
            # flight, so the HAM clock gate reaches 8/8 (2.4 GHz) before the
            # real matmul stream (else ~35 of them run at half clock) ----
            warm_l = cpool.tile([128, 1], bf16)
            nc.vector.memset(warm_l[:], 0.0)
            warm_r = cpool.tile([128, 512], bf16)
            nc.vector.memset(warm_r[:], 0.0)
            pwarm = psum.tile([1, 512], f32)
            for _ in range(7):
                nc.tensor.matmul(
                    pwarm[:], warm_l[:], warm_r[:], start=True, stop=True
                )

            # ---- v = dinv @ x : 96 accumulating matmuls, dinv stationary ----
            pv = psum.tile([1, F_IN], f32)
            q = 0
            for g, rsz in enumerate(X_GROUPS):
                for r in range(rsz):
                    nc.tensor.matmul(
                        pv[:],
                        dinvT[:, x_offs[g] + r : x_offs[g] + r + 1],
                        xg[g][:, r, :],
                        start=(q == 0),
                        stop=(q == R_TOT - 1),
                        skip_group_check=True,
                    )
                    q += 1

            # v row -> column via TensorE transpose, cast bf16 for the agg mm
            vrow = cpool.tile([1, F_IN], f32)
            nc.vector.tensor_copy(vrow[:], pv[:])
            pvcol = psum.tile([F_IN, 1], f32)
            nc.tensor.transpose(pvcol[:], vrow[:], one_s[:])
            vcolc = cpool.tile([F_IN, 1], bf16)
            nc.vector.tensor_copy(vcolc[:], pvcol[:])
            vcol = vcolc[:]

            # A2[p, o] = agg[o] = sum_j v[j] W[j, o]   (v bcast as lhsT)
            pA2 = psum.tile([128, F_OUT], f32)
            nc.tensor.matmul(
                pA2[:],
                vcol.broadcast_to([F_IN, 128]),
                w_s[:],
                start=True,
                stop=True,
            )
            A2 = cpool.tile([128, F_OUT], bf16)
            nc.vector.tensor_copy(A2[:], pA2[:])
            if not zero_bias:
                pB2 = psum.tile([128, F_OUT], f32)
                nc.tensor.matmul(
                    pB2[:], ones_row[:], bias_s[:], start=True, stop=True
                )
                B2 = cpool.tile([128, F_OUT], bf16)
                nc.vector.tensor_copy(B2[:], pB2[:])

            # ---- out[p, n, :] = dinvS[p, n] * A2 (+ bias) ----
            # rows 0..7 on DVE (tensor_scalar 4x mode), rows 8..11 on ScalarE
            # (activation copy-with-scale); out groups ride the idle sync queue
            og_rows = [(0, 5), (5, 9), (9, 12)]
            for g, (a, b) in enumerate(og_rows):
                og = opool.tile([128, b - a, F_OUT], bf16, tag=f"og{g}",
                                name=f"og{g}")
                for j in range(b - a):
                    i = a + j
                    if zero_bias:
                        if i < DVE_ROWS:
                            nc.vector.tensor_scalar_mul(
                                og[:, j, :], A2[:], dinvSf[:, i : i + 1]
                            )
                        else:
                            nc.scalar.activation(
                                og[:, j, :],
                                A2[:],
                                mybir.ActivationFunctionType.Copy,
                                scale=dinvSf[:, i : i + 1],
                            )
                    else:
                        nc.vector.scalar_tensor_tensor(
                            og[:, j, :],
                            A2[:],
                            dinvSf[:, i : i + 1],
                            B2[:],
                            op0=mybir.AluOpType.mult,
                            op1=mybir.AluOpType.add,
                        )
                # rows 8-11 are ACT-computed; their write rides the scalar
                # ring (issued after the ACT copies), the rest ride sync
                oeng = nc.scalar if a >= DVE_ROWS else nc.sync
                oeng.dma_start(out_view[:, a:b, :], og[:])

    nc.compile()
    return nc


def _get_nc(zero_bias):
    key = ("nc", zero_bias)
    if key not in _cache:
        _cache[key] = _build_nc(zero_bias)
    return _cache[key]


def _host_dinv(edge_index: np.ndarray) -> np.ndarray:
    """Exact deduplicated symmetric degree -> 1/sqrt(deg), matching
    adj[a,b]=1; adj[b,a]=1; deg=adj.sum(1)."""
    a = edge_index[0].astype(np.int64)
    b = edge_index[1].astype(np.int64)
    keys = np.unique(np.concatenate([a * N + b, b * N + a]))
    deg = np.bincount(keys // N, minlength=N).astype(np.float32)
    with np.errstate(divide="ignore"):
        dinv = (np.float32(1.0) / np.sqrt(deg)).astype(np.float32)
    return dinv


def kernel(x, edge_index, weight, bias, _trace=False):
    from concourse import bass_utils

    x = np.ascontiguousarray(x, dtype=np.float32)
    weight = np.ascontiguousarray(weight, dtype=np.float32)
    bias = np.ascontiguousarray(bias, dtype=np.float32)
    dinv = _host_dinv(np.asarray(edge_index))

    zero_bias = bool(np.all(bias == 0.0))
    nc = _get_nc(zero_bias)

    import ml_dtypes

    bf16 = ml_dtypes.bfloat16
    xp = np.zeros((N_PAD, F_IN), bf16)
    xp[:N] = x.astype(bf16)
    dp = np.zeros((N_PAD,), np.float32)
    dp[:N] = dinv
    dinvT = dp.reshape(128, R_TOT)          # dinvT[p, r] = dinv[p*96+r]

    w16 = weight.astype(bf16)
    in_maps = []
    for c in range(N_CORES):
        r0 = c * ROWS
        ds = np.zeros((ROWS_PAD,), np.float32)
        ds[:ROWS] = dinv[r0 : r0 + ROWS]
        dinvSf = np.ascontiguousarray(ds.reshape(128, NT_OUT))
        cb16 = np.ascontiguousarray(
            np.concatenate([dinvT, dinvSf], axis=1)
        ).astype(bf16)
        im = {
            "x": xp,
            "cb16": cb16,
            "dinvSf": dinvSf,
            "weight": w16,
        }
        if not zero_bias:
            im["bias"] = bias
        in_maps.append(im)

    res = bass_utils.run_bass_kernel_spmd(
        nc, in_maps, core_ids=list(range(N_CORES)), trace=_trace
    )
    out = np.concatenate(
        [
            res.results[c]["out"][:ROWS].astype(np.float32)
            for c in range(N_CORES)
        ],
        axis=0,
    )
    if _trace:
        _cache["last_results"] = res
    return out


# revision 26
# speedup vs baseline: 1.0016x; 1.0016x over previous
"""GCNConv (rank-1 normalized aggregation) Trainium2 kernel, SPMD over 8 cores.

Math (faithful to the torch/jax reference):
    h    = x @ W
    adj  = symmetric 0/1 adjacency from edge_index (duplicates collapse: SET, not add)
    deg  = adj.sum(1);  dinv = 1/sqrt(deg)
    agg  = dinv @ h                      # rank-1 identity, [F_OUT]
    out  = dinv[:, None] * agg[None, :] + bias

Since agg = (dinv @ x) @ W, h is never materialized:
    v    = dinv @ x            ([F_IN] weighted row-sum)
    agg  = v @ W               (TensorE)
    out_c = dinv_c (x) agg + bias     (rows sharded across cores)

Collectives here have a ~55us fixed latency, far above the 8-core floor, so
every core reads the full x (3.07MB bf16, ~9us at HBM BW) and computes v
locally; only the O(N*F_OUT) output is sharded.

v runs entirely on TensorE: per 128-node slice r, the dinv column [128,1] is
the stationary operand (LDWEIGHTS cost scales with stationary *columns*, so a
1-column load is ~1 cycle) and the raw bf16 x slice [128,128] is the moving
operand; all 96 matmuls accumulate into one [1,128] PSUM tile. No DVE
pre-multiply, no fat ones-matmul.

DMA shape discipline (measured): each 128-descriptor dma_start costs ~680ns
of HWDGE sequencer issue time and each transfer's completion semaphore fires
~1.5-2.5us after its last byte (receipt latency), so x travels in 6 big
r-grouped transfers (24KB contiguous per partition total), ALL on the sync
ring in consumption order (so sems fire in the order TensorE consumes),
with small first group (early TensorE start) and small last group (short
post-receipt matmul backlog). Consts ride the scalar ring. ~3us of dummy
matmuls pre-warm the PE HAM clock gate to 8/8 during the DMA lead-in.
Measured fixed costs: ~7us engine preamble before the first DMA issue and
~3.5us semaphore teardown after the last write - a trivial kernel measures
13.4us on this stack, so ~29us total = floor + 9us x stream + tails.

The exact deduplicated degree (an integer/sorting problem, not a flops
problem) is computed on host with np.unique; all O(N*F) floating-point work
runs on the NeuronCores. Output travels bf16 and is upcast on host.
"""

import numpy as np

N, F_IN, F_OUT = 12000, 128, 256
N_CORES = 8
ROWS = N // N_CORES            # 1500 output rows per core
NT_OUT = 12                    # rows per partition in the output shard
ROWS_PAD = NT_OUT * 128        # 1536
R_TOT = 96                     # x rows per partition (node = p*96 + r)
N_PAD = 128 * R_TOT            # 12288
X_GROUPS = [4, 8, 12, 24, 28, 8, 4, 4, 4]
DVE_ROWS = 8                  # expansion rows on DVE (tensor_scalar_mul)
ACT_ROWS = NT_OUT - DVE_ROWS   # expansion rows on ScalarE (activation scale)
C_DT = R_TOT + NT_OUT          # packed bf16 consts: dinvT | dinvS

_cache = {}


def _build_nc(zero_bias):
    import concourse.bacc as bacc
    import concourse.mybir as mybir
    import concourse.tile as tile

    f32 = mybir.dt.float32
    bf16 = mybir.dt.bfloat16

    nc = bacc.Bacc(
        "TRN2",
        target_bir_lowering=False,
        debug=False,
        num_devices=N_CORES,
    )

    # x padded to [12288, 128] bf16; partition p holds rows p*96 .. p*96+95
    x_d = nc.dram_tensor("x", [N_PAD, F_IN], bf16, kind="ExternalInput")
    # cb16[:, 0:96] = dinvT (dinvT[p, r] = dinv[p*96+r]);
    # cb16[:, 96:108] = dinvS (dinvS[p, i] = dinv[core*1500 + p*12 + i])
    cb16_d = nc.dram_tensor("cb16", [128, C_DT], bf16, kind="ExternalInput")
    # f32 copy of dinvS for the ScalarE activation scale operand
    dinvSf_d = nc.dram_tensor("dinvSf", [128, NT_OUT], f32, kind="ExternalInput")
    w_d = nc.dram_tensor("weight", [F_IN, F_OUT], bf16, kind="ExternalInput")
    if not zero_bias:
        b_d = nc.dram_tensor("bias", [F_OUT], f32, kind="ExternalInput")
    out_d = nc.dram_tensor("out", [ROWS_PAD, F_OUT], bf16, kind="ExternalOutput")

    x_prm = x_d.ap().rearrange("(p r) m -> p r m", p=128)       # [128,96,128]
    # out row p*12 + n  ->  partition p, free n  (natural row-major)
    out_view = out_d.ap().rearrange("(p n) m -> p n m", p=128)  # [128,12,256]

    with tile.TileContext(nc) as tc:
        with (
            tc.tile_pool(name="const", bufs=1) as cpool,
            tc.tile_pool(name="xbuf", bufs=1) as xpool,
            tc.tile_pool(name="obuf", bufs=1) as opool,
            tc.tile_pool(name="ps", bufs=1, space="PSUM") as psum,
        ):
            # ---- DMAs: x groups alternate the two HWDGE queues; the packed
            # consts lead the scalar queue so TensorE can start on group 0 ----
            cb16 = cpool.tile([128, C_DT], bf16)
            nc.scalar.dma_start(cb16[:], cb16_d.ap())
            dinvT = cb16[:, 0:R_TOT]
            dinvS = cb16[:, R_TOT : R_TOT + NT_OUT]

            xg = []
            r0 = 0
            x_offs = []
            for g, rsz in enumerate(X_GROUPS):
                t = xpool.tile([128, rsz, F_IN], bf16, tag=f"xg{g}",
                               name=f"xg{g}")
                # all x on the sync ring, in consumption order: bytes (and so
                # completion sems) arrive exactly in the order TensorE needs
                nc.sync.dma_start(t[:], x_prm[:, r0 : r0 + rsz, :])
                xg.append(t)
                x_offs.append(r0)
                r0 += rsz

            dinvSf = cpool.tile([128, NT_OUT], f32)
            nc.scalar.dma_start(dinvSf[:], dinvSf_d.ap())
            w_s = cpool.tile([F_IN, F_OUT], bf16)
            nc.scalar.dma_start(w_s[:], w_d.ap())
            if not zero_bias:
                bias_s = cpool.tile([1, F_OUT], f32)
                nc.scalar.dma_start(
                    bias_s[:], b_d.ap().rearrange("(a n) -> a n", a=1)
                )
                ones_row = cpool.tile([1, 128], f32)
                nc.vector.memset(ones_row[:], 1.0)
            one_s = cpool.tile([1, 1], f32)
            nc.vector.memset(one_s[:], 1.0)

            # ---- PE pre-warm: ~3us of dummy matmuls while the x DMA is in
            # flight, so the HAM clock gate reaches 8/8 (2.4 GHz) before the
            # real matmul stream (else ~35 of them run at half clock) ----
            warm_l = cpool.tile([128, 1], bf16)
            nc.vector.memset(warm_l[:], 0.0)
            warm_r = cpool.tile([128, 512], bf16)
            nc.vector.memset(warm_r[:], 0.0)
            pwarm = psum.tile([1, 512], f32)
            for _ in range(7):
                nc.tensor.matmul(
                    pwarm[:], warm_l[:], warm_r[:], start=True, stop=True
                )

            # ---- v = dinv @ x : 96 accumulating matmuls, dinv stationary ----
            pv = psum.tile([1, F_IN], f32)
            q = 0
            for g, rsz in enumerate(X_GROUPS):
                if g in (1, 2):
                    # bridge matmuls: hold the PE HAM gate warm across the
                    # early DMA-sem bubbles (each <3.4us but jittery)
                    for _ in range(3 - g):
                        nc.tensor.matmul(pwarm[:], warm_l[:], warm_r[:],
                                         start=True, stop=True)
                for r in range(rsz):
                    nc.tensor.matmul(
                        pv[:],
                        dinvT[:, x_offs[g] + r : x_offs[g] + r + 1],
                        xg[g][:, r, :],
                        start=(q == 0),
                        stop=(q == R_TOT - 1),
                        skip_group_check=True,
                    )
                    q += 1

            # v row -> column via TensorE transpose, cast bf16 for the agg mm
            vrow = cpool.tile([1, F_IN], f32)
            nc.vector.tensor_copy(vrow[:], pv[:])
            pvcol = psum.tile([F_IN, 1], f32)
            nc.tensor.transpose(pvcol[:], vrow[:], one_s[:])
            vcolc = cpool.tile([F_IN, 1], bf16)
            nc.vector.tensor_copy(vcolc[:], pvcol[:])
            vcol = vcolc[:]

            # A2[p, o] = agg[o] = sum_j v[j] W[j, o]   (v bcast as lhsT)
            pA2 = psum.tile([128, F_OUT], f32)
            nc.tensor.matmul(
                pA2[:],
                vcol.broadcast_to([F_IN, 128]),
                w_s[:],
                start=True,
                stop=True,
            )
            A2 = cpool.tile([128, F_OUT], bf16)
            nc.vector.tensor_copy(A2[:], pA2[:])
            if not zero_bias:
                pB2 = psum.tile([128, F_OUT], f32)
                nc.tensor.matmul(
                    pB2[:], ones_row[:], bias_s[:], start=True, stop=True
                )
                B2 = cpool.tile([128, F_OUT], bf16)
                nc.vector.tensor_copy(B2[:], pB2[:])

            # ---- out[p, n, :] = dinvS[p, n] * A2 (+ bias) ----
            # rows 0..7 on DVE (tensor_scalar 4x mode), rows 8..11 on ScalarE
            # (activation copy-with-scale); out groups ride the idle sync queue
            og_rows = [(0, 4), (4, 8), (8, 12)]
            for g, (a, b) in enumerate(og_rows):
                og = opool.tile([128, b - a, F_OUT], bf16, tag=f"og{g}",
                                name=f"og{g}")
                for j in range(b - a):
                    i = a + j
                    if zero_bias:
                        if i < DVE_ROWS:
                            nc.vector.tensor_scalar_mul(
                                og[:, j, :], A2[:], dinvSf[:, i : i + 1]
                            )
                        else:
                            nc.scalar.activation(
                                og[:, j, :],
                                A2[:],
                                mybir.ActivationFunctionType.Copy,
                                scale=dinvSf[:, i : i + 1],
                            )
                    else:
                        nc.vector.scalar_tensor_tensor(
                            og[:, j, :],
                            A2[:],
                            dinvSf[:, i : i + 1],
                            B2[:],
                            op0=mybir.AluOpType.mult,
                            op1=mybir.AluOpType.add,
                        )
                # rows 8-11 are ACT-computed; their write rides the scalar
                # ring (issued after the ACT copies), the rest ride sync
                oeng = nc.scalar if a >= DVE_ROWS else nc.sync
                oeng.dma_start(out_view[:, a:b, :], og[:])

    nc.compile()
    return nc


def _get_nc(zero_bias):
    key = ("nc", zero_bias)
    if key not in _cache:
        _cache[key] = _build_nc(zero_bias)
    return _cache[key]


def _host_dinv(edge_index: np.ndarray) -> np.ndarray:
    """Exact deduplicated symmetric degree -> 1/sqrt(deg), matching
    adj[a,b]=1; adj[b,a]=1; deg=adj.sum(1)."""
    a = edge_index[0].astype(np.int64)
    b = edge_index[1].astype(np.int64)
    keys = np.unique(np.concatenate([a * N + b, b * N + a]))
    deg = np.bincount(keys // N, minlength=N).astype(np.float32)
    with np.errstate(divide="ignore"):
        dinv = (np.float32(1.0) / np.sqrt(deg)).astype(np.float32)
    return dinv


def kernel(x, edge_index, weight, bias, _trace=False):
    from concourse import bass_utils

    x = np.ascontiguousarray(x, dtype=np.float32)
    weight = np.ascontiguousarray(weight, dtype=np.float32)
    bias = np.ascontiguousarray(bias, dtype=np.float32)
    dinv = _host_dinv(np.asarray(edge_index))

    zero_bias = bool(np.all(bias == 0.0))
    nc = _get_nc(zero_bias)

    import ml_dtypes

    bf16 = ml_dtypes.bfloat16
    xp = np.zeros((N_PAD, F_IN), bf16)
    xp[:N] = x.astype(bf16)
    dp = np.zeros((N_PAD,), np.float32)
    dp[:N] = dinv
    dinvT = dp.reshape(128, R_TOT)          # dinvT[p, r] = dinv[p*96+r]

    w16 = weight.astype(bf16)
    in_maps = []
    for c in range(N_CORES):
        r0 = c * ROWS
        ds = np.zeros((ROWS_PAD,), np.float32)
        ds[:ROWS] = dinv[r0 : r0 + ROWS]
        dinvSf = np.ascontiguousarray(ds.reshape(128, NT_OUT))
        cb16 = np.ascontiguousarray(
            np.concatenate([dinvT, dinvSf], axis=1)
        ).astype(bf16)
        im = {
            "x": xp,
            "cb16": cb16,
            "dinvSf": dinvSf,
            "weight": w16,
        }
        if not zero_bias:
            im["bias"] = bias
        in_maps.append(im)

    res = bass_utils.run_bass_kernel_spmd(
        nc, in_maps, core_ids=list(range(N_CORES)), trace=_trace
    )
    out = np.concatenate(
        [
            res.results[c]["out"][:ROWS].astype(np.float32)
            for c in range(N_CORES)
        ],
        axis=0,
    )
    if _trace:
        _cache["last_results"] = res
    return out
